# revision 1
# baseline (speedup 1.0000x reference)
"""AdaPT int8-quantized 3x3 conv (systolic, exact) on 8 TRN2 NeuronCores.

Full inputs: x [32,8,384,384] f32, weight [8,8,3,3] f32, bias [8] f32.
Sharding: data-parallel over batch (4 images per core), amax all-reduced
(max) across cores, weights/bias replicated.

Per-core plan:
  - load x in two 192-row halves into a wide [128, 49, 384] SBUF layout
    (partition = (q, img, ci), q = 49-row block), junk rows zeroed
  - DVE abs-max reduce + gpsimd partition all-reduce -> local amax;
    AllReduce(max) over the 8 cores via a DRAM bounce buffer
  - quantize wide with the fp32 magic-number round (bit-exact RNE, same
    as jnp.round), output bf16 (ints <= 127 are exact in bf16)
  - SBUF->SBUF DMA builds a dx-shifted x3-replicated rhs chunk
    [128, 34, 384] (partition = 32*img + ci*3 + dx)
  - conv: per 512-wide PSUM bank, 4 column-band matmul tiles
    (tile_position (32c, 32c), one image per band, concurrent) x 3
    accumulating dy-matmuls with row-shifted rhs windows; weights are
    bf16 slices of a per-band-replicated stationary tensor
  - evacuate PSUM [128,512] full-width: out = psum * (1/(sx*sw)) + bias
  - DMA the 32 useful partitions (4 img x 8 co) to HBM

All long-lived SBUF buffers use static allocations (alloc_sbuf_tensor):
the tile-pool allocator reuses slots by inferred lifetime and was
observed overlapping long-lived tiles.
"""

import numpy as np

N_CORES = 8
IMG = 4          # images per core
CI = 8
CO = 8
H = W = 384
HALF = 192       # rows per half
QROWS = 49       # rows per partition block (4 blocks cover HALF + halo)
CHUNK = 32       # output rows per rhs chunk
NCHUNK = HALF // CHUNK   # 6
NBANK = CHUNK * W // 512  # 24 psum banks (512 cols) per chunk
MAGIC = 12582912.0  # 1.5 * 2**23, fp32 round-to-nearest-int trick
MAX_Q = 127.0

_cached = {}


def _build(n_cores=N_CORES, debug=False):
    from concourse import bacc, bass, tile, mybir, bass_isa

    f32 = mybir.dt.float32
    bf16 = mybir.dt.bfloat16

    nc = bacc.Bacc(
        "TRN2", target_bir_lowering=False, debug=debug, num_devices=n_cores
    )

    x_ext = nc.declare_dram_parameter("x", [IMG, CI, H, W], f32, isOutput=False)
    w_ext = nc.declare_dram_parameter("weight", [CO, CI, 3, 3], f32, isOutput=False)
    b_ext = nc.declare_dram_parameter("bias", [CO], f32, isOutput=False)
    out_ext = nc.declare_dram_parameter("out", [IMG, CO, H, W], f32, isOutput=True)
    dbg_ext = nc.declare_dram_parameter("dbg", [128, 8], f32, isOutput=True)

    # ---- static SBUF buffers (long-lived) ----
    sb = lambda name, shape, dt: nc.alloc_sbuf_tensor(name, list(shape), dt).ap()
    xh = sb("xh_s", [128, QROWS, W], f32)
    qxh = sb("qxh_s", [128, QROWS, W], bf16)
    rep_bufs = [sb(f"rep{t}_s", [128, CHUNK + 2, W], bf16) for t in range(2)]
    w24 = sb("w24_s", [24, 3 * CO], f32)
    qw = sb("qw_s", [128, 48], bf16)
    aw = sb("aw_s", [24, 1], f32)
    aw_all = sb("awall_s", [24, 1], f32)
    sw = sb("sw_s", [24, 1], f32)
    bias_e = sb("biase_s", [128, 1], f32)
    ax = sb("ax_s", [128, 1], f32)
    ax_t = sb("axt_s", [128, 1], f32)
    ax_all = sb("axall_s", [128, 1], f32)
    axg = sb("axg_s", [128, 1], f32)
    sx = sb("sx_s", [128, 1], f32)
    aw128 = sb("aw128_s", [128, 1], f32)
    inv = sb("inv_s", [128, 1], f32)

    import itertools
    _q = itertools.cycle((0, 1, 2))

    def next_eng():
        return (nc.sync, nc.gpsimd, nc.scalar)[next(_q)]

    with tile.TileContext(nc) as tc:
        with (
            tc.tile_pool(name="stage", bufs=2) as spool,
            tc.tile_pool(name="psum", bufs=8, space="PSUM") as pspool,
            tc.tile_pool(name="dram", bufs=1, space="DRAM") as dpool,
        ):
            # ---------------- weight prep ----------------
            # qw layout (per 32-partition band c, replicated):
            #   qw[32c + kx*8 + ci, 0:8]    = dy0 weights, cols 8:32 zero
            #   qw[32c + kx*8 + ci, 32:40]  = dy1, [40:48] = dy2
            # (dx-major so all DMAs touch contiguous partition ranges)
            with nc.allow_non_contiguous_dma(reason="one-time 576-elem w load"):
                for ky in range(3):
                    for kx in range(3):
                        dst = w24[8 * kx:8 * kx + 8, CO * ky:CO * ky + CO]
                        src = w_ext[:, :, ky, kx].rearrange("co ci -> ci co")
                        nc.sync.dma_start(out=dst, in_=src)

            nc.vector.tensor_reduce(
                aw[:, :], w24[:, :], mybir.AxisListType.X, mybir.AluOpType.max,
                apply_absolute_value=True,
            )
            nc.gpsimd.partition_all_reduce(
                aw_all[:, :], aw[:, :], channels=24, reduce_op=bass_isa.ReduceOp.max
            )
            nc.vector.reciprocal(sw[:, :], aw_all[:, :])
            nc.vector.tensor_scalar(
                out=sw[:, :], in0=sw[:, :], scalar1=MAX_Q, scalar2=None,
                op0=mybir.AluOpType.mult,
            )
            # quantize weights: round(w * sw) via magic, to bf16
            nc.vector.tensor_scalar(
                out=w24[:, :], in0=w24[:, :], scalar1=sw[:, :], scalar2=MAGIC,
                op0=mybir.AluOpType.mult, op1=mybir.AluOpType.add,
            )
            nc.vector.memset(qw[:, :], 0.0)
            nc.scalar.activation(
                qw[0:24, 0:CO], w24[:, 0:CO],
                mybir.ActivationFunctionType.Copy, bias=-MAGIC, scale=1.0,
            )
            nc.scalar.activation(
                qw[0:24, 32:32 + 2 * CO], w24[:, CO:3 * CO],
                mybir.ActivationFunctionType.Copy, bias=-MAGIC, scale=1.0,
            )
            for c in range(1, IMG):
                nc.sync.dma_start(out=qw[32 * c:32 * c + 24, :], in_=qw[0:24, :])

            # bias vector on evac partitions: p = 32*img + co
            nc.vector.memset(bias_e[:, :], 0.0)
            for c in range(IMG):
                nc.sync.dma_start(out=bias_e[32 * c:32 * c + CO, :], in_=b_ext[:])

            # rhs double buffers, zeroed once: pad partitions and the dx
            # edge columns stay zero forever
            for rb in rep_bufs:
                nc.vector.memset(rb[:, :, :], 0.0)

            # ---------------- x amax pass ----------------
            for h in range(2):
                _load_half(nc, xh, x_ext, h, next_eng)
                nc.vector.tensor_reduce(
                    ax_t[:, :], xh[:, :, :], mybir.AxisListType.XY,
                    mybir.AluOpType.max, apply_absolute_value=True,
                )
                if h == 0:
                    nc.vector.tensor_copy(ax[:, :], ax_t[:, :])
                else:
                    nc.vector.tensor_tensor(
                        out=ax[:, :], in0=ax[:, :], in1=ax_t[:, :],
                        op=mybir.AluOpType.max,
                    )
            nc.gpsimd.partition_all_reduce(
                ax_all[:, :], ax[:, :], channels=128, reduce_op=bass_isa.ReduceOp.max
            )

            # ---------------- amax all-reduce across cores ----------------
            cc_in = dpool.tile([1, 128], f32)
            cc_out = dpool.tile([1, 128], f32)
            nc.sync.dma_start(
                out=cc_in.rearrange("one p -> p one"), in_=ax_all[:, :]
            )
            nc.gpsimd.collective_compute(
                "AllReduce",
                mybir.AluOpType.max,
                replica_groups=[list(range(n_cores))],
                ins=[cc_in.opt()],
                outs=[cc_out.opt()],
            )
            nc.sync.dma_start(
                out=axg[:, :], in_=cc_out.rearrange("one p -> p one")
            )

            # sx = 127/axg  (per-partition, all equal)
            nc.vector.reciprocal(sx[:, :], axg[:, :])
            nc.vector.tensor_scalar(
                out=sx[:, :], in0=sx[:, :], scalar1=MAX_Q, scalar2=None,
                op0=mybir.AluOpType.mult,
            )
            # inv = axg * aw / 127^2
            nc.gpsimd.partition_broadcast(aw128[:, :], aw_all[0:1, :])
            nc.vector.tensor_tensor(
                out=inv[:, :], in0=axg[:, :], in1=aw128[:, :],
                op=mybir.AluOpType.mult,
            )
            nc.vector.tensor_scalar(
                out=inv[:, :], in0=inv[:, :], scalar1=1.0 / (MAX_Q * MAX_Q),
                scalar2=None, op0=mybir.AluOpType.mult,
            )

            # debug: scale-chain intermediates
            dbg = nc.alloc_sbuf_tensor("dbg_s", [128, 8], f32).ap()
            nc.vector.memset(dbg[:, :], 0.0)
            nc.vector.tensor_copy(dbg[:, 0:1], ax[:, :])
            nc.vector.tensor_copy(dbg[:, 1:2], ax_all[:, :])
            nc.vector.tensor_copy(dbg[:, 2:3], axg[:, :])
            nc.vector.tensor_copy(dbg[:, 3:4], sx[:, :])
            nc.vector.tensor_copy(dbg[:, 4:5], inv[:, :])
            nc.vector.tensor_copy(dbg[:, 5:6], aw128[:, :])
            nc.vector.tensor_copy(dbg[0:24, 6:7], aw_all[:, :])
            nc.vector.tensor_copy(dbg[0:24, 7:8], sw[:, :])
            nc.sync.dma_start(out=dbg_ext[:, :], in_=dbg[:, :])

            # ---------------- main loop over halves ----------------
            out_flat = out_ext.rearrange("i co h w -> i co (h w)")
            for h in (1, 0):
                if h == 0:
                    _load_half(nc, xh, x_ext, 0, next_eng)
                # quantize wide: round(x*sx) -> bf16
                nc.vector.tensor_scalar(
                    out=xh[:, :, :], in0=xh[:, :, :], scalar1=sx[:, :],
                    scalar2=MAGIC, op0=mybir.AluOpType.mult,
                    op1=mybir.AluOpType.add,
                )
                nc.scalar.activation(
                    qxh[:, :, :], xh[:, :, :], mybir.ActivationFunctionType.Copy,
                    bias=-MAGIC, scale=1.0,
                )

                for r in range(NCHUNK):
                    # rhs chunk: rep[32*img + dx*8 + ci, rr, xo]
                    #   = qx[img, ci, h*192 + 32*r - 1 + rr, xo + dx - 1]
                    rep = rep_bufs[((1 - h) * NCHUNK + r) % 2]
                    b0 = CHUNK * r  # buffer row of chunk start (y - 1)
                    pieces = []
                    bb = b0
                    while bb < b0 + CHUNK + 2:
                        q = bb // QROWS
                        n = min((q + 1) * QROWS, b0 + CHUNK + 2) - bb
                        pieces.append((q, bb - q * QROWS, bb - b0, n))
                        bb += n
                    rep_eng = next_eng()
                    for i in range(IMG):
                        for dx in range(3):
                            xs, xe = max(0, 1 - dx), W - max(0, dx - 1)
                            for (q, qr, rr, n) in pieces:
                                p0 = 32 * q + 8 * i
                                d0 = 32 * i + 8 * dx
                                rep_eng.dma_start(
                                    out=rep[d0:d0 + 8, rr:rr + n, xs:xe],
                                    in_=qxh[p0:p0 + 8, qr:qr + n,
                                            xs + dx - 1:xe + dx - 1],
                                )

                    rep_f = rep.rearrange("p r x -> p (r x)")
                    st = None
                    for wb in range(NBANK):
                        ps = pspool.tile([128, 512], f32, tag="ps")
                        for dy in range(3):
                            for c in range(IMG):
                                off = dy * W + wb * 512
                                if dy == 0:
                                    # M=32: cols 8:32 are zero weights so
                                    # pad psum partitions get written zeros
                                    lhsT = qw[32 * c:32 * c + 24, 0:32]
                                    out_ap = ps[32 * c:32 * c + 32, :]
                                else:
                                    lhsT = qw[32 * c:32 * c + 24,
                                              24 + CO * dy:24 + CO * dy + CO]
                                    out_ap = ps[32 * c:32 * c + CO, :]
                                nc.tensor.matmul(
                                    out_ap,
                                    lhsT,
                                    rep_f[32 * c:32 * c + 24, off:off + 512],
                                    start=(dy == 0),
                                    stop=(dy == 2),
                                    skip_group_check=True,
                                    tile_position=(32 * c, 32 * c),
                                )
                        # batch 8 psum banks into one staging tile; one
                        # output DMA per image per group, on the ACT queue
                        g = wb % 8
                        if g == 0:
                            st = spool.tile([128, 8 * 512], f32, tag="st")
                        if wb % 2 == 0:
                            nc.vector.tensor_scalar(
                                out=st[:, 512 * g:512 * g + 512], in0=ps[:, :],
                                scalar1=inv[:, :], scalar2=bias_e[:, :],
                                op0=mybir.AluOpType.mult,
                                op1=mybir.AluOpType.add,
                            )
                        else:
                            # out = Identity(psum * inv + bias) on ScalarE
                            nc.scalar.activation(
                                st[:, 512 * g:512 * g + 512], ps[:, :],
                                mybir.ActivationFunctionType.Identity,
                                bias=bias_e[:, :], scale=inv[:, :],
                            )
                        if g == 7:
                            off_out = ((h * HALF + CHUNK * r) * W
                                       + (wb - 7) * 512)
                            for i in range(IMG):
                                next_eng().dma_start(
                                    out=out_flat[i, :, off_out:off_out + 4096],
                                    in_=st[32 * i:32 * i + CO, :],
                                )

    nc.compile()
    return nc


def _load_half(nc, xh, x_ext, h, next_eng):
    """Load rows so that xh[q*32 + i*8 + c, rr, :] = x[i, c, y, :] with
    y = h*192 - 1 + q*49 + rr.  Junk rows (y < 0 or y > 383) zeroed.
    One DMA covers all 4 images (32 contiguous partitions); rows are
    split into <=25-row pieces to keep AP dims under the 16k-elem cap."""
    def load_q(q, r0, nrows, y0):
        r = 0
        while r < nrows:
            n = min(25, nrows - r)
            next_eng().dma_start(
                out=xh[32 * q:32 * q + 32, r0 + r:r0 + r + n, :],
                in_=x_ext[:, :, y0 + r:y0 + r + n, :],
            )
            r += n

    if h == 0:
        nc.vector.memset(xh[0:32, 0:1, :], 0.0)
        load_q(0, 1, 48, 0)
        for q in range(1, 4):
            load_q(q, 0, 49, q * 49 - 1)
    else:
        for q in range(3):
            load_q(q, 0, 49, 191 + q * 49)
        load_q(3, 0, 46, 338)
        nc.vector.memset(xh[96:128, 46:49, :], 0.0)


def _get_nc():
    if "nc" not in _cached:
        _cached["nc"] = _build()
    return _cached["nc"]


def kernel(x, weight, bias):
    from concourse.bass_utils import run_bass_kernel_spmd

    nc = _get_nc()
    in_maps = [
        {
            "x": np.ascontiguousarray(x[i * IMG:(i + 1) * IMG], dtype=np.float32),
            "weight": np.ascontiguousarray(weight, dtype=np.float32),
            "bias": np.ascontiguousarray(bias, dtype=np.float32),
        }
        for i in range(N_CORES)
    ]
    res = run_bass_kernel_spmd(nc, in_maps, core_ids=list(range(N_CORES)))
    out = np.concatenate([res.results[i]["out"] for i in range(N_CORES)], axis=0)
    return out.astype(np.float32)



# revision 5
# speedup vs baseline: 5.4447x; 5.4447x over previous
"""AdaPT int8-quantized 3x3 conv (exact) on 8 TRN2 NeuronCores.

Full inputs: x [32,8,384,384] f32, weight [8,8,3,3] f32, bias [8] f32.

Sharding: batch x height grid (2 batch-halves x 4 row-strips of 96 rows).
Each core gets x_core [16, 8, 98, 384] (rows pre-padded with the +-1 halo
on the host, zeros at image edges) and writes a padded output stream
out_core [16, 8, 96*386]; the host strips the 2 pad columns per row.
amax is per-tensor: local abs-max, then AllReduce(max) across cores.

Per-core plan (partition p = 8*img + ci everywhere):
  - load x strip in two ~half row-blocks into xh [128, 50, 384] f32,
    abs-max reduce each loaded piece (DVE/Pool split), AllReduce(max)
  - quantize with the fp32 magic-number round (bit-exact RNE) into a
    PADDED bf16 stream qxh [128, 98, 386] (col 0/385 zero): all nine
    3x3 taps become stream offsets ky*386 + kx - 1 into this buffer
  - conv: per 512-wide PSUM bank, 9 accumulating matmuls with
    block-diagonal weights [128 K, 128 M] (K = 16 img x 8 ci,
    M = 16 img x 8 co) -- one matmul per tap covers all 16 images
  - evacuate psum*inv + bias into an 8-bank staging tile, one
    contiguous 2 MB DMA per group into the padded out stream
"""

import numpy as np

N_CORES = 8
IMG = 16         # images per core
CI = 8
CO = 8
H = W = 384
WP = W + 2       # padded row width in the qx / out streams
ROWS = 96        # output rows per core strip
RH = ROWS + 2    # input rows incl halo
STREAM = ROWS * WP            # 37056, padded out stream length
S_BEG, S_END = 1, STREAM - 1  # real out positions [1, 37055)
NBANK = -(-(S_END - S_BEG) // 512)  # 73 (72 full + one 190)
GROUP = 8        # banks per staging tile / out DMA
MAGIC = 12582912.0  # 1.5 * 2**23, fp32 round-to-nearest-int trick
MAX_Q = 127.0

_cached = {}


def _build(n_cores=N_CORES, debug=False):
    from concourse import bacc, bass, tile, mybir, bass_isa

    f32 = mybir.dt.float32
    bf16 = mybir.dt.bfloat16

    nc = bacc.Bacc(
        "TRN2", target_bir_lowering=False, debug=debug, num_devices=n_cores
    )

    x_ext = nc.declare_dram_parameter("x", [IMG, CI, RH, W], f32, isOutput=False)
    w_ext = nc.declare_dram_parameter("weight", [CO, CI, 3, 3], f32, isOutput=False)
    b_ext = nc.declare_dram_parameter("bias", [CO], f32, isOutput=False)
    out_ext = nc.declare_dram_parameter("out", [IMG, CO, STREAM], f32, isOutput=True)

    # ---- static SBUF buffers (long-lived) ----
    sb = lambda name, shape, dt: nc.alloc_sbuf_tensor(name, list(shape), dt).ap()
    xh = sb("xh_s", [128, 50, W], f32)           # f32 staging (one half)
    qxh = sb("qxh_s", [128, RH, WP], bf16)       # padded quantized stream
    w24 = sb("w24_s", [24, 3 * CO], f32)         # w[co,ci,ky,kx] @ [8kx+ci, 8ky+co]
    w24q = sb("w24q_s", [24, 3, CO], bf16)
    qw_t = sb("qwt_s", [CI, 9, CO], bf16)        # [ci, g=3ky+kx, co]
    qwbig = sb("qwbig_s", [128, 9, 128], bf16)   # block-diag lhsT per tap
    aw = sb("aw_s", [24, 1], f32)
    aw_all = sb("awall_s", [24, 1], f32)
    sw = sb("sw_s", [24, 1], f32)
    bias_e = sb("biase_s", [128, 1], f32)
    axp = [sb(f"axp{j}_s", [128, 1], f32) for j in range(4)]
    ax = sb("ax_s", [128, 1], f32)
    ax_all = sb("axall_s", [128, 1], f32)
    axg = sb("axg_s", [128, 1], f32)
    sx = sb("sx_s", [128, 1], f32)
    aw128 = sb("aw128_s", [128, 1], f32)
    inv = sb("inv_s", [128, 1], f32)

    qxh_f = qxh.rearrange("p r c -> p (r c)")
    qwbig_f = qwbig.rearrange("p g m -> p (g m)")

    # load piece: xh[:, r0:r0+n, :] = x_core[:, :, src0:src0+n, :]
    def load_piece(eng, r0, src0, n):
        eng.dma_start(
            out=xh[:, r0:r0 + n, :], in_=x_ext[:, :, src0:src0 + n, :]
        )

    with tile.TileContext(nc) as tc:
        with (
            tc.tile_pool(name="stage", bufs=2) as spool,
            tc.tile_pool(name="psum", bufs=8, space="PSUM") as pspool,
            tc.tile_pool(name="dram", bufs=1, space="DRAM") as dpool,
        ):
            # ---------------- weight prep (ACT/DVE queues, off x-load path) --
            with nc.allow_non_contiguous_dma(reason="one-time 576-elem w load"):
                for ky in range(3):
                    for kx in range(3):
                        dst = w24[8 * kx:8 * kx + 8, CO * ky:CO * ky + CO]
                        src = w_ext[:, :, ky, kx].rearrange("co ci -> ci co")
                        nc.scalar.dma_start(out=dst, in_=src)

            nc.vector.tensor_reduce(
                aw[:, :], w24[:, :], mybir.AxisListType.X, mybir.AluOpType.max,
                apply_absolute_value=True,
            )
            nc.gpsimd.partition_all_reduce(
                aw_all[:, :], aw[:, :], channels=24, reduce_op=bass_isa.ReduceOp.max
            )
            nc.vector.reciprocal(sw[:, :], aw_all[:, :])
            nc.vector.tensor_scalar(
                out=sw[:, :], in0=sw[:, :], scalar1=MAX_Q, scalar2=None,
                op0=mybir.AluOpType.mult,
            )
            # quantize weights: round(w * sw) via magic, to bf16
            nc.vector.tensor_scalar(
                out=w24[:, :], in0=w24[:, :], scalar1=sw[:, :], scalar2=MAGIC,
                op0=mybir.AluOpType.mult, op1=mybir.AluOpType.add,
            )
            nc.scalar.activation(
                w24q.rearrange("p a b -> p (a b)"), w24[:, :],
                mybir.ActivationFunctionType.Copy, bias=-MAGIC, scale=1.0,
            )
            # qw_t[ci, 3ky+kx, co] = w24q[8kx+ci, ky, co]
            with nc.allow_non_contiguous_dma(reason="one-time w rearrange"):
                for kx in range(3):
                    nc.scalar.dma_start(
                        out=qw_t[:, kx::3, :],
                        in_=w24q[8 * kx:8 * kx + 8, :, :],
                    )
                nc.vector.memset(qwbig[:, :, :], 0.0)
                for i in range(IMG):
                    nc.scalar.dma_start(
                        out=qwbig[8 * i:8 * i + 8, :, 8 * i:8 * i + 8],
                        in_=qw_t[:, :, :],
                    )

            # bias vector on evac partitions p = 8*img + co: log-doubling
            nc.scalar.dma_start(out=bias_e[0:CO, :], in_=b_ext[:])
            for m in (8, 16, 32, 64):
                nc.scalar.dma_start(out=bias_e[m:2 * m, :], in_=bias_e[0:m, :])

            # zero the pad columns of the qx stream (quantize never writes them)
            nc.vector.memset(qxh[:, :, 0:1], 0.0)
            nc.vector.memset(qxh[:, :, WP - 1:WP], 0.0)

            # ---------------- x amax pass ----------------
            # pieces: (xh row0, x_core row0, nrows); halves h1 = rows 50:98,
            # h0 = rows 0:50.  h1 first so h0 stays resident for quantize.
            h1_pieces = [(0, 50, 24), (24, 74, 24)]
            h0_pieces = [(0, 0, 25), (25, 25, 25)]
            for j, (r0, s0, n) in enumerate(h1_pieces + h0_pieces):
                load_piece(nc.sync, r0, s0, n)
                nc.vector.tensor_reduce(
                    axp[j][:, :], xh[:, r0:r0 + n, :], mybir.AxisListType.XY,
                    mybir.AluOpType.max, apply_absolute_value=True,
                )
            nc.vector.tensor_tensor(
                out=axp[0][:, :], in0=axp[0][:, :], in1=axp[1][:, :],
                op=mybir.AluOpType.max,
            )
            nc.vector.tensor_tensor(
                out=axp[2][:, :], in0=axp[2][:, :], in1=axp[3][:, :],
                op=mybir.AluOpType.max,
            )
            nc.vector.tensor_tensor(
                out=ax[:, :], in0=axp[0][:, :], in1=axp[2][:, :],
                op=mybir.AluOpType.max,
            )
            nc.gpsimd.partition_all_reduce(
                ax_all[:, :], ax[:, :], channels=128, reduce_op=bass_isa.ReduceOp.max
            )

            # ---------------- amax all-reduce across cores ----------------
            cc_in = dpool.tile([1, 128], f32)
            cc_out = dpool.tile([1, 128], f32)
            nc.sync.dma_start(
                out=cc_in.rearrange("one p -> p one"), in_=ax_all[:, :]
            )
            nc.gpsimd.collective_compute(
                "AllReduce",
                mybir.AluOpType.max,
                replica_groups=[list(range(n_cores))],
                ins=[cc_in.opt()],
                outs=[cc_out.opt()],
            )
            nc.sync.dma_start(
                out=axg[:, :], in_=cc_out.rearrange("one p -> p one")
            )

            # sx = 127/axg  (per-partition, all equal)
            nc.vector.reciprocal(sx[:, :], axg[:, :])
            nc.vector.tensor_scalar(
                out=sx[:, :], in0=sx[:, :], scalar1=MAX_Q, scalar2=None,
                op0=mybir.AluOpType.mult,
            )
            # inv = axg * aw / 127^2
            nc.gpsimd.partition_broadcast(aw128[:, :], aw_all[0:1, :])
            nc.vector.tensor_tensor(
                out=inv[:, :], in0=axg[:, :], in1=aw128[:, :],
                op=mybir.AluOpType.mult,
            )
            nc.vector.tensor_scalar(
                out=inv[:, :], in0=inv[:, :], scalar1=1.0 / (MAX_Q * MAX_Q),
                scalar2=None, op0=mybir.AluOpType.mult,
            )

            # ---------------- quantize ----------------
            # DVE: xh = xh*sx + MAGIC (in place); ACT: qxh = Copy(xh - MAGIC)
            def quant(r0, n, q0):
                nc.vector.tensor_scalar(
                    out=xh[:, r0:r0 + n, :], in0=xh[:, r0:r0 + n, :],
                    scalar1=sx[:, :], scalar2=MAGIC,
                    op0=mybir.AluOpType.mult, op1=mybir.AluOpType.add,
                )
                nc.scalar.activation(
                    qxh[:, q0 + r0:q0 + r0 + n, 1:W + 1], xh[:, r0:r0 + n, :],
                    mybir.ActivationFunctionType.Copy, bias=-MAGIC, scale=1.0,
                )

            for (r0, s0, n) in h0_pieces:        # h0 resident in xh
                quant(r0, n, 0)
            for (r0, s0, n) in h1_pieces:        # reload h1, then quantize
                load_piece(nc.sync, r0, s0, n)
                quant(r0, n, 50)

            # ---------------- conv: 9 taps x 73 banks ----------------
            evac_cycle = (nc.scalar, nc.vector)
            bank = 0
            g_out = 0
            while bank < NBANK:
                nb = min(GROUP, NBANK - bank)
                glen = sum(
                    min(512, S_END - (S_BEG + 512 * (bank + k))) for k in range(nb)
                )
                st = spool.tile([128, GROUP * 512], f32, tag="st")
                for k in range(nb):
                    b = bank + k
                    s_a = S_BEG + 512 * b
                    N = min(512, S_END - s_a)
                    ps = pspool.tile([128, 512], f32, tag="ps")
                    for g9 in range(9):
                        ky, kx = divmod(g9, 3)
                        off = s_a + ky * WP + kx - 1
                        nc.tensor.matmul(
                            ps[:, 0:N],
                            qwbig_f[:, 128 * g9:128 * g9 + 128],
                            qxh_f[:, off:off + N],
                            start=(g9 == 0),
                            stop=(g9 == 8),
                        )
                    eng = evac_cycle[b % 2]
                    if eng is nc.scalar:
                        nc.scalar.activation(
                            st[:, 512 * k:512 * k + N], ps[:, 0:N],
                            mybir.ActivationFunctionType.Identity,
                            bias=bias_e[:, :], scale=inv[:, :],
                        )
                    else:
                        eng.tensor_scalar(
                            out=st[:, 512 * k:512 * k + N], in0=ps[:, 0:N],
                            scalar1=inv[:, :], scalar2=bias_e[:, :],
                            op0=mybir.AluOpType.mult, op1=mybir.AluOpType.add,
                        )
                s_g = S_BEG + 512 * bank
                nc.sync.dma_start(
                    out=out_ext[:, :, s_g:s_g + glen], in_=st[:, 0:glen]
                )
                bank += nb
                g_out += 1

    nc.compile()
    return nc


def _get_nc():
    if "nc" not in _cached:
        _cached["nc"] = _build()
    return _cached["nc"]


def make_core_inputs(x, weight, bias):
    """Shard full inputs into per-core input maps (host side)."""
    x = np.ascontiguousarray(x, dtype=np.float32)
    weight = np.ascontiguousarray(weight, dtype=np.float32)
    bias = np.ascontiguousarray(bias, dtype=np.float32)
    in_maps = []
    for core in range(N_CORES):
        b, h = divmod(core, 4)
        xc = np.zeros((IMG, CI, RH, W), dtype=np.float32)
        lo = 96 * h - 1
        src_lo, src_hi = max(lo, 0), min(lo + RH, H)
        xc[:, :, src_lo - lo:src_hi - lo, :] = (
            x[IMG * b:IMG * b + IMG, :, src_lo:src_hi, :]
        )
        in_maps.append({"x": xc, "weight": weight, "bias": bias})
    return in_maps


def assemble_output(results):
    """Gather per-core padded streams into the full output."""
    out = np.empty((2 * IMG, CO, H, W), dtype=np.float32)
    for core in range(N_CORES):
        b, h = divmod(core, 4)
        strip = results[core]["out"].reshape(IMG, CO, ROWS, WP)[:, :, :, 1:W + 1]
        out[IMG * b:IMG * b + IMG, :, 96 * h:96 * h + ROWS, :] = strip
    return out


def kernel(x, weight, bias):
    from concourse.bass_utils import run_bass_kernel_spmd

    nc = _get_nc()
    in_maps = make_core_inputs(x, weight, bias)
    res = run_bass_kernel_spmd(nc, in_maps, core_ids=list(range(N_CORES)))
    return assemble_output(res.results)


# revision 11
# speedup vs baseline: 7.3523x; 1.3503x over previous
"""AdaPT int8-quantized 3x3 conv (exact) on 8 TRN2 NeuronCores.

Full inputs: x [32,8,384,384] f32, weight [8,8,3,3] f32, bias [8] f32.

Sharding: batch x height grid (2 batch-halves x 4 row-strips of 96 rows).
Each core gets x_core [16, 8, 98, 384] (rows pre-padded with the +-1 halo
on the host, zeros at image edges) and writes a padded output stream
out_core [16, 8, 96*386]; the host strips the 2 pad columns per row.
amax is per-tensor: local abs-max, then AllReduce(max) across cores.

Per-core plan (partition p = 8*img + ci everywhere):
  - load x in 12-row pieces spread over 4 DMA queues (SP/ACT/Pool/DVE) so
    transfers overlap; abs-max each piece as it lands (DVE X-reduce and
    Pool XYZWC-reduce split the work); AllReduce(max) across cores
  - quantize with the fp32 magic-number round (bit-exact RNE) into a
    PADDED bf16 stream qxh [128, 98, 386] (col 0/385 zero): all nine
    3x3 taps become stream offsets ky*386 + kx - 1 into this buffer
  - conv: per 512-wide PSUM bank, 9 accumulating matmuls with
    block-diagonal weights [128 K, 128 M] (K = 16 img x 8 ci,
    M = 16 img x 8 co) -- one matmul per tap covers all 16 images
  - evacuate psum*inv + bias into an 8-bank staging tile, one
    contiguous 2 MB DMA per group into the padded out stream
  - x is read 1.5x: the h0 half (rows 0:50) stays resident in xh f32
    for quantize; the h1 half (rows 50:98) is reloaded during quant-h0
"""

import numpy as np

N_CORES = 8
IMG = 16         # images per core
CI = 8
CO = 8
H = W = 384
WP = W + 2       # padded row width in the qx / out streams
ROWS = 96        # output rows per core strip
RH = ROWS + 2    # input rows incl halo
STREAM = ROWS * WP            # 37056, padded out stream length
S_BEG, S_END = 1, STREAM - 1  # real out positions [1, 37055)
NBANK = -(-(S_END - S_BEG) // 512)  # 73 (72 full + one 190)
GROUP = 8        # banks per staging tile / out DMA
MAGIC = 12582912.0  # 1.5 * 2**23, fp32 round-to-nearest-int trick
MAX_Q = 127.0

_cached = {}


def _build(n_cores=N_CORES, debug=False):
    from concourse import bacc, bass, tile, mybir, bass_isa

    f32 = mybir.dt.float32
    bf16 = mybir.dt.bfloat16

    nc = bacc.Bacc(
        "TRN2", target_bir_lowering=False, debug=debug, num_devices=n_cores
    )

    x_ext = nc.declare_dram_parameter("x", [IMG, CI, RH, W], f32, isOutput=False)
    w_ext = nc.declare_dram_parameter("weight", [CO, CI, 3, 3], f32, isOutput=False)
    b_ext = nc.declare_dram_parameter("bias", [CO], f32, isOutput=False)
    out_ext = nc.declare_dram_parameter("out", [IMG, CO, STREAM], f32, isOutput=True)

    # ---- static SBUF buffers (long-lived) ----
    sb = lambda name, shape, dt: nc.alloc_sbuf_tensor(name, list(shape), dt).ap()
    xh = sb("xh_s", [128, 50, W], f32)           # f32 staging (one half)
    qxh = sb("qxh_s", [128, RH, WP], bf16)       # padded quantized stream
    w24 = sb("w24_s", [24, 3 * CO], f32)         # w[co,ci,ky,kx] @ [8kx+ci, 8ky+co]
    w24q = sb("w24q_s", [24, 3, CO], bf16)
    qw_t = sb("qwt_s", [CI, 9, CO], bf16)        # [ci, g=3ky+kx, co]
    qwbig = sb("qwbig_s", [128, 9, 128], bf16)   # block-diag lhsT per tap
    aw = sb("aw_s", [24, 1], f32)
    aw_all = sb("awall_s", [24, 1], f32)
    sw = sb("sw_s", [24, 1], f32)
    bias_e = sb("biase_s", [128, 1], f32)
    axd = [sb(f"axd{j}_s", [128, 1], f32) for j in range(4)]  # DVE partials
    axp = [sb(f"axp{j}_s", [1, 1], f32) for j in range(4)]    # Pool partials
    ax0 = sb("ax0_s", [1, 1], f32)
    ax_all = sb("axall_s", [128, 1], f32)
    axg = sb("axg_s", [128, 1], f32)
    axg8 = sb("axg8_s", [1, n_cores], f32)
    axg0 = sb("axg0_s", [1, 1], f32)
    sx = sb("sx_s", [128, 1], f32)
    aw128 = sb("aw128_s", [128, 1], f32)
    inv = sb("inv_s", [128, 1], f32)

    qxh_f = qxh.rearrange("p r c -> p (r c)")
    qwbig_f = qwbig.rearrange("p g m -> p (g m)")

    # load piece: xh[:, r0:r0+n, :] = x_core[:, :, src0:src0+n, :]
    def load_piece(eng, r0, src0, n):
        eng.dma_start(
            out=xh[:, r0:r0 + n, :], in_=x_ext[:, :, src0:src0 + n, :]
        )

    with tile.TileContext(nc) as tc:
        with (
            tc.tile_pool(name="stage", bufs=2) as spool,
            tc.tile_pool(name="psum", bufs=8, space="PSUM") as pspool,
            tc.tile_pool(name="dram", bufs=1, space="DRAM") as dpool,
        ):
            # ---------------- x loads + amax, 4-queue parallel -------------
            # piece: (xh row0, x_core row0, nrows); phase1 = h1 rows 50:98,
            # phase2 = h0 rows 0:50 (stays resident for quantize).
            phase1 = [(0, 50, 12), (12, 62, 12), (24, 74, 12), (36, 86, 12)]
            phase2 = [(0, 0, 12), (12, 12, 12), (24, 24, 12), (36, 36, 14)]
            q1 = (nc.sync, nc.scalar, nc.gpsimd, nc.sync)
            q2 = (nc.scalar, nc.gpsimd, nc.sync, nc.scalar)

            nd = np_ = 0

            def amax_piece(j, r0, n):
                nonlocal nd, np_
                if j % 2 == 0:   # DVE
                    nc.vector.tensor_reduce(
                        axd[nd][:, :], xh[:, r0:r0 + n, :],
                        mybir.AxisListType.XY, mybir.AluOpType.max,
                        apply_absolute_value=True,
                    )
                    nd += 1
                else:            # Pool all-axis reduce -> [1,1]
                    nc.gpsimd.tensor_reduce(
                        axp[np_][:, :], xh[:, r0:r0 + n, :],
                        mybir.AxisListType.XYZWC, mybir.AluOpType.max,
                        apply_absolute_value=True,
                    )
                    np_ += 1

            for j, (r0, s0, n) in enumerate(phase1):
                load_piece(q1[j], r0, s0, n)
                amax_piece(j, r0, n)
            for j, (r0, s0, n) in enumerate(phase2):
                load_piece(q2[j], r0, s0, n)
                amax_piece(j, r0, n)

            # ---------------- combine amax partials ----------------
            nc.vector.tensor_tensor(
                out=axd[0][:, :], in0=axd[0][:, :], in1=axd[1][:, :],
                op=mybir.AluOpType.max,
            )
            nc.vector.tensor_tensor(
                out=axd[2][:, :], in0=axd[2][:, :], in1=axd[3][:, :],
                op=mybir.AluOpType.max,
            )
            nc.vector.tensor_tensor(
                out=axd[0][:, :], in0=axd[0][:, :], in1=axd[2][:, :],
                op=mybir.AluOpType.max,
            )
            nc.vector.tensor_tensor(
                out=axp[0][:, :], in0=axp[0][:, :], in1=axp[1][:, :],
                op=mybir.AluOpType.max,
            )
            nc.vector.tensor_tensor(
                out=axp[2][:, :], in0=axp[2][:, :], in1=axp[3][:, :],
                op=mybir.AluOpType.max,
            )
            nc.vector.tensor_tensor(
                out=axp[0][:, :], in0=axp[0][:, :], in1=axp[2][:, :],
                op=mybir.AluOpType.max,
            )
            nc.gpsimd.partition_all_reduce(
                ax_all[:, :], axd[0][:, :], channels=128,
                reduce_op=bass_isa.ReduceOp.max,
            )
            nc.vector.tensor_tensor(
                out=ax0[:, :], in0=ax_all[0:1, :], in1=axp[0][:, :],
                op=mybir.AluOpType.max,
            )
            # ------------- amax exchange: AllGather + local max -------------
            cc_in = dpool.tile([1, 1], f32)
            cc_out = dpool.tile([1, n_cores], f32)
            nc.sync.dma_start(out=cc_in[:, :], in_=ax0[:, :])
            nc.gpsimd.collective_compute(
                "AllGather",
                mybir.AluOpType.bypass,
                replica_groups=[list(range(n_cores))],
                ins=[cc_in.opt()],
                outs=[cc_out.opt()],
            )
            nc.sync.dma_start(out=axg8[:, :], in_=cc_out[:, :])

            # ---------------- weight prep (scalar queue, off load path) ----
            with nc.allow_non_contiguous_dma(reason="one-time 576-elem w load"):
                for ky in range(3):
                    for kx in range(3):
                        dst = w24[8 * kx:8 * kx + 8, CO * ky:CO * ky + CO]
                        src = w_ext[:, :, ky, kx].rearrange("co ci -> ci co")
                        nc.scalar.dma_start(out=dst, in_=src)

            nc.vector.tensor_reduce(
                aw[:, :], w24[:, :], mybir.AxisListType.X, mybir.AluOpType.max,
                apply_absolute_value=True,
            )
            nc.gpsimd.partition_all_reduce(
                aw_all[:, :], aw[:, :], channels=24, reduce_op=bass_isa.ReduceOp.max
            )
            nc.vector.reciprocal(sw[:, :], aw_all[:, :])
            nc.vector.tensor_scalar(
                out=sw[:, :], in0=sw[:, :], scalar1=MAX_Q, scalar2=None,
                op0=mybir.AluOpType.mult,
            )
            # quantize weights: round(w * sw) via magic, to bf16
            nc.vector.tensor_scalar(
                out=w24[:, :], in0=w24[:, :], scalar1=sw[:, :], scalar2=MAGIC,
                op0=mybir.AluOpType.mult, op1=mybir.AluOpType.add,
            )
            nc.scalar.activation(
                w24q.rearrange("p a b -> p (a b)"), w24[:, :],
                mybir.ActivationFunctionType.Copy, bias=-MAGIC, scale=1.0,
            )
            # qw_t[ci, 3ky+kx, co] = w24q[8kx+ci, ky, co]
            with nc.allow_non_contiguous_dma(reason="one-time w rearrange"):
                for kx in range(3):
                    nc.scalar.dma_start(
                        out=qw_t[:, kx::3, :],
                        in_=w24q[8 * kx:8 * kx + 8, :, :],
                    )
                nc.vector.memset(qwbig[:, :, :], 0.0)
                for i in range(IMG):
                    nc.scalar.dma_start(
                        out=qwbig[8 * i:8 * i + 8, :, 8 * i:8 * i + 8],
                        in_=qw_t[:, :, :],
                    )

            # bias vector on evac partitions p = 8*img + co: log-doubling
            nc.scalar.dma_start(out=bias_e[0:CO, :], in_=b_ext[:])
            for m in (8, 16, 32, 64):
                nc.scalar.dma_start(out=bias_e[m:2 * m, :], in_=bias_e[0:m, :])

            # zero the pad columns of the qx stream (quantize never writes them)
            nc.vector.memset(qxh[:, :, 0:1], 0.0)
            nc.vector.memset(qxh[:, :, WP - 1:WP], 0.0)

            nc.vector.tensor_reduce(
                axg0[:, :], axg8[:, :], mybir.AxisListType.X,
                mybir.AluOpType.max,
            )
            nc.gpsimd.partition_broadcast(axg[:, :], axg0[:, :])

            # sx = 127/axg  (per-partition, all equal)
            nc.vector.reciprocal(sx[:, :], axg[:, :])
            nc.vector.tensor_scalar(
                out=sx[:, :], in0=sx[:, :], scalar1=MAX_Q, scalar2=None,
                op0=mybir.AluOpType.mult,
            )
            # inv = axg * aw / 127^2
            nc.gpsimd.partition_broadcast(aw128[:, :], aw_all[0:1, :])
            nc.vector.tensor_tensor(
                out=inv[:, :], in0=axg[:, :], in1=aw128[:, :],
                op=mybir.AluOpType.mult,
            )
            nc.vector.tensor_scalar(
                out=inv[:, :], in0=inv[:, :], scalar1=1.0 / (MAX_Q * MAX_Q),
                scalar2=None, op0=mybir.AluOpType.mult,
            )

            # ---------------- quantize ----------------
            # DVE: xh = xh*sx + MAGIC (in place); ACT: qxh = Copy(xh - MAGIC)
            def quant(r0, n, q0):
                nc.vector.tensor_scalar(
                    out=xh[:, r0:r0 + n, :], in0=xh[:, r0:r0 + n, :],
                    scalar1=sx[:, :], scalar2=MAGIC,
                    op0=mybir.AluOpType.mult, op1=mybir.AluOpType.add,
                )
                nc.scalar.activation(
                    qxh[:, q0 + r0:q0 + r0 + n, 1:W + 1], xh[:, r0:r0 + n, :],
                    mybir.ActivationFunctionType.Copy, bias=-MAGIC, scale=1.0,
                )

            for (r0, n) in ((0, 6), (6, 22), (28, 22)):   # h0 resident in xh
                quant(r0, n, 0)
            # reload h1 on two queues, quantize as pieces land
            load_piece(nc.sync, 0, 50, 24)
            quant(0, 24, 50)
            load_piece(nc.gpsimd, 24, 74, 24)
            quant(24, 24, 50)

            # ---------------- conv: 9 taps x 73 banks ----------------
            evac_cycle = (nc.scalar, nc.vector)
            out_q = (nc.sync, nc.gpsimd)
            bank = 0
            g_out = 0
            group_sizes = [GROUP] * 8 + [5, 3, 1]
            assert sum(group_sizes) == NBANK
            while bank < NBANK:
                nb = group_sizes[g_out]
                glen = sum(
                    min(512, S_END - (S_BEG + 512 * (bank + k))) for k in range(nb)
                )
                st = spool.tile([128, GROUP * 512], f32, tag="st")
                for k in range(nb):
                    b = bank + k
                    s_a = S_BEG + 512 * b
                    N = min(512, S_END - s_a)
                    ps = pspool.tile([128, 512], f32, tag="ps")
                    for g9 in range(9):
                        ky, kx = divmod(g9, 3)
                        off = s_a + ky * WP + kx - 1
                        nc.tensor.matmul(
                            ps[:, 0:N],
                            qwbig_f[:, 128 * g9:128 * g9 + 128],
                            qxh_f[:, off:off + N],
                            start=(g9 == 0),
                            stop=(g9 == 8),
                        )
                    eng = evac_cycle[b % 2]
                    if eng is nc.scalar:
                        nc.scalar.activation(
                            st[:, 512 * k:512 * k + N], ps[:, 0:N],
                            mybir.ActivationFunctionType.Identity,
                            bias=bias_e[:, :], scale=inv[:, :],
                        )
                    else:
                        eng.tensor_scalar(
                            out=st[:, 512 * k:512 * k + N], in0=ps[:, 0:N],
                            scalar1=inv[:, :], scalar2=bias_e[:, :],
                            op0=mybir.AluOpType.mult, op1=mybir.AluOpType.add,
                        )
                s_g = S_BEG + 512 * bank
                out_q[g_out % 2].dma_start(
                    out=out_ext[:, :, s_g:s_g + glen], in_=st[:, 0:glen]
                )
                bank += nb
                g_out += 1

    nc.compile()
    return nc


def _get_nc():
    if "nc" not in _cached:
        _cached["nc"] = _build()
    return _cached["nc"]


def make_core_inputs(x, weight, bias):
    """Shard full inputs into per-core input maps (host side)."""
    x = np.ascontiguousarray(x, dtype=np.float32)
    weight = np.ascontiguousarray(weight, dtype=np.float32)
    bias = np.ascontiguousarray(bias, dtype=np.float32)
    in_maps = []
    for core in range(N_CORES):
        b, h = divmod(core, 4)
        xc = np.zeros((IMG, CI, RH, W), dtype=np.float32)
        lo = 96 * h - 1
        src_lo, src_hi = max(lo, 0), min(lo + RH, H)
        xc[:, :, src_lo - lo:src_hi - lo, :] = (
            x[IMG * b:IMG * b + IMG, :, src_lo:src_hi, :]
        )
        in_maps.append({"x": xc, "weight": weight, "bias": bias})
    return in_maps


def assemble_output(results):
    """Gather per-core padded streams into the full output."""
    out = np.empty((2 * IMG, CO, H, W), dtype=np.float32)
    for core in range(N_CORES):
        b, h = divmod(core, 4)
        strip = results[core]["out"].reshape(IMG, CO, ROWS, WP)[:, :, :, 1:W + 1]
        out[IMG * b:IMG * b + IMG, :, 96 * h:96 * h + ROWS, :] = strip
    return out


def kernel(x, weight, bias):
    from concourse.bass_utils import run_bass_kernel_spmd

    nc = _get_nc()
    in_maps = make_core_inputs(x, weight, bias)
    res = run_bass_kernel_spmd(nc, in_maps, core_ids=list(range(N_CORES)))
    return assemble_output(res.results)


# revision 19
# speedup vs baseline: 7.6038x; 1.0342x over previous
"""AdaPT int8-quantized 3x3 conv (exact) on 8 TRN2 NeuronCores.

Full inputs: x [32,8,384,384] f32, weight [8,8,3,3] f32, bias [8] f32.

Sharding: batch x height grid (2 batch-halves x 4 row-strips of 96 rows).
Each core gets x_core [16, 8, 98, 384] (rows pre-padded with the +-1 halo
on the host, zeros at image edges) and writes a padded output stream
out_core [16, 8, 96*386]; the host strips the 2 pad columns per row.
amax is per-tensor: local abs-max, then AllReduce(max) across cores.

Per-core plan (partition p = 8*img + ci everywhere):
  - load x in 8-row pieces spread over 3 DMA queues (SP/ACT/Pool) so
    transfers overlap; abs-max each piece as it lands (DVE XY-reduce and
    Pool XYZWC-reduce split the work); AllGather + local max across cores
  - quantize with the fp32 magic-number round (bit-exact RNE) into a
    PADDED bf16 stream qxh [128, 98, 386] (col 0/385 zero): all nine
    3x3 taps become stream offsets ky*386 + kx - 1 into this buffer
  - conv: per 512-wide PSUM bank, 9 accumulating matmuls with
    block-diagonal weights [128 K, 128 M] (K = 16 img x 8 ci,
    M = 16 img x 8 co) -- one matmul per tap covers all 16 images
  - evacuate psum*inv + bias into an 8-bank staging tile, one
    contiguous 2 MB DMA per group into the padded out stream
  - x is read 1.5x: the h0 half (rows 0:50) stays resident in xh f32
    for quantize; the h1 half (rows 50:98) is reloaded during quant-h0
"""

import numpy as np

N_CORES = 8
IMG = 16         # images per core
CI = 8
CO = 8
H = W = 384
WP = W + 2       # padded row width in the qx / out streams
ROWS = 96        # output rows per core strip
RH = ROWS + 2    # input rows incl halo
STREAM = ROWS * WP            # 37056, padded out stream length
S_BEG, S_END = 1, STREAM - 1  # real out positions [1, 37055)
NBANK = -(-(S_END - S_BEG) // 512)  # 73 (72 full + one 190)
GROUP = 8        # banks per staging tile / out DMA
MAGIC = 12582912.0  # 1.5 * 2**23, fp32 round-to-nearest-int trick
MAX_Q = 127.0

_cached = {}


def _build(n_cores=N_CORES, debug=False):
    from concourse import bacc, bass, tile, mybir, bass_isa

    f32 = mybir.dt.float32
    bf16 = mybir.dt.bfloat16

    nc = bacc.Bacc(
        "TRN2", target_bir_lowering=False, debug=debug, num_devices=n_cores
    )

    x_ext = nc.declare_dram_parameter("x", [IMG, CI, RH, W], f32, isOutput=False)
    w_ext = nc.declare_dram_parameter("weight", [CO, CI, 3, 3], f32, isOutput=False)
    b_ext = nc.declare_dram_parameter("bias", [CO], f32, isOutput=False)
    out_ext = nc.declare_dram_parameter("out", [IMG, CO, STREAM], f32, isOutput=True)

    # ---- static SBUF buffers (long-lived) ----
    sb = lambda name, shape, dt: nc.alloc_sbuf_tensor(name, list(shape), dt).ap()
    xh = sb("xh_s", [128, 50, W], f32)           # f32 staging (one half)
    qxh = sb("qxh_s", [128, RH, WP], bf16)       # padded quantized stream
    w24 = sb("w24_s", [24, 3 * CO], f32)         # w[co,ci,ky,kx] @ [8kx+ci, 8ky+co]
    w24q = sb("w24q_s", [24, 3, CO], bf16)
    qw_t = sb("qwt_s", [CI, 9, CO], bf16)        # [ci, g=3ky+kx, co]
    qwbig = sb("qwbig_s", [128, 9, 128], bf16)   # block-diag lhsT per tap
    aw = sb("aw_s", [24, 1], f32)
    aw_all = sb("awall_s", [24, 1], f32)
    sw = sb("sw_s", [24, 1], f32)
    bias_e = sb("biase_s", [128, 1], f32)
    axd = [sb(f"axd{j}_s", [128, 1], f32) for j in range(6)]  # DVE partials
    axp = [sb(f"axp{j}_s", [1, 1], f32) for j in range(7)]    # Pool partials
    ax0 = sb("ax0_s", [1, 1], f32)
    ax_all = sb("axall_s", [128, 1], f32)
    axg = sb("axg_s", [128, 1], f32)
    axg8 = sb("axg8_s", [1, n_cores], f32)
    axg0 = sb("axg0_s", [1, 1], f32)
    sx = sb("sx_s", [128, 1], f32)
    aw128 = sb("aw128_s", [128, 1], f32)
    inv = sb("inv_s", [128, 1], f32)

    qxh_f = qxh.rearrange("p r c -> p (r c)")
    qwbig_f = qwbig.rearrange("p g m -> p (g m)")

    # load piece: xh[:, r0:r0+n, :] = x_core[:, :, src0:src0+n, :]
    def load_piece(eng, r0, src0, n):
        eng.dma_start(
            out=xh[:, r0:r0 + n, :], in_=x_ext[:, :, src0:src0 + n, :]
        )

    with tile.TileContext(nc) as tc:
        with (
            tc.tile_pool(name="stage", bufs=2) as spool,
            tc.tile_pool(name="psum", bufs=8, space="PSUM") as pspool,
            tc.tile_pool(name="dram", bufs=1, space="DRAM") as dpool,
        ):
            # ---------------- x loads + amax, 3-queue parallel -------------
            # 8-row pieces; phase1 = h1 rows 50:98 (xh rows 0:48), phase2 =
            # h0 rows 0:50 (stays resident in xh for quantize).  Phase-2
            # piece k reuses phase-1 piece k's xh rows, so the WAR chain per
            # pair is load->amax->load->amax; small pieces keep it short.
            # Phase-2 loads go on a different queue than their phase-1 twin.
            qs = (nc.sync, nc.scalar, nc.gpsimd)
            nd = np_ = 0

            def amax_piece(j, r0, n):
                nonlocal nd, np_
                if j % 2 == 0:   # DVE
                    nc.vector.tensor_reduce(
                        axd[nd][:, :], xh[:, r0:r0 + n, :],
                        mybir.AxisListType.XY, mybir.AluOpType.max,
                        apply_absolute_value=True,
                    )
                    nd += 1
                else:            # Pool all-axis reduce -> [1,1]
                    nc.gpsimd.tensor_reduce(
                        axp[np_][:, :], xh[:, r0:r0 + n, :],
                        mybir.AxisListType.XYZWC, mybir.AluOpType.max,
                        apply_absolute_value=True,
                    )
                    np_ += 1

            for j in range(6):                    # phase 1: h1 rows
                r0 = 8 * j
                load_piece(qs[j % 3], r0, 50 + r0, 8)
                amax_piece(j, r0, 8)
            # rows 48:50 overlap no phase-1 piece: load early, off-chain
            load_piece(nc.scalar, 48, 48, 2)
            amax_piece(1, 48, 2)                  # Pool
            # phase-2 loads stay off the gpsimd queue: SWDGE desc-gen runs on
            # the Pool engine and head-of-line blocks the Pool amax reduces
            for j in range(6):                    # phase 2: h0 rows
                r0 = 8 * j
                load_piece(qs[j % 2], r0, r0, 8)
                amax_piece(j, r0, 8)

            # ---------------- combine amax partials ----------------
            # sequential folds: each runs as soon as its piece lands, so
            # only the final fold trails the last amax
            for k in range(1, nd):
                nc.vector.tensor_tensor(
                    out=axd[0][:, :], in0=axd[0][:, :], in1=axd[k][:, :],
                    op=mybir.AluOpType.max,
                )
            for k in range(1, np_):
                nc.vector.tensor_tensor(
                    out=axp[0][:, :], in0=axp[0][:, :], in1=axp[k][:, :],
                    op=mybir.AluOpType.max,
                )
            nc.gpsimd.partition_all_reduce(
                ax_all[:, :], axd[0][:, :], channels=128,
                reduce_op=bass_isa.ReduceOp.max,
            )
            nc.vector.tensor_tensor(
                out=ax0[:, :], in0=ax_all[0:1, :], in1=axp[0][:, :],
                op=mybir.AluOpType.max,
            )
            # ------------- amax exchange: AllGather + local max -------------
            cc_in = dpool.tile([1, 1], f32)
            cc_out = dpool.tile([1, n_cores], f32)
            nc.sync.dma_start(out=cc_in[:, :], in_=ax0[:, :])
            nc.gpsimd.collective_compute(
                "AllGather",
                mybir.AluOpType.bypass,
                replica_groups=[list(range(n_cores))],
                ins=[cc_in.opt()],
                outs=[cc_out.opt()],
            )
            nc.sync.dma_start(out=axg8[:, :], in_=cc_out[:, :])

            # ---------------- weight prep (scalar queue, off load path) ----
            with nc.allow_non_contiguous_dma(reason="one-time 576-elem w load"):
                for ky in range(3):
                    for kx in range(3):
                        dst = w24[8 * kx:8 * kx + 8, CO * ky:CO * ky + CO]
                        src = w_ext[:, :, ky, kx].rearrange("co ci -> ci co")
                        nc.scalar.dma_start(out=dst, in_=src)

            nc.vector.tensor_reduce(
                aw[:, :], w24[:, :], mybir.AxisListType.X, mybir.AluOpType.max,
                apply_absolute_value=True,
            )
            nc.gpsimd.partition_all_reduce(
                aw_all[:, :], aw[:, :], channels=24, reduce_op=bass_isa.ReduceOp.max
            )
            nc.vector.reciprocal(sw[:, :], aw_all[:, :])
            nc.vector.tensor_scalar(
                out=sw[:, :], in0=sw[:, :], scalar1=MAX_Q, scalar2=None,
                op0=mybir.AluOpType.mult,
            )
            # quantize weights: round(w * sw) via magic, to bf16
            nc.vector.tensor_scalar(
                out=w24[:, :], in0=w24[:, :], scalar1=sw[:, :], scalar2=MAGIC,
                op0=mybir.AluOpType.mult, op1=mybir.AluOpType.add,
            )
            nc.scalar.activation(
                w24q.rearrange("p a b -> p (a b)"), w24[:, :],
                mybir.ActivationFunctionType.Copy, bias=-MAGIC, scale=1.0,
            )
            # qw_t[ci, 3ky+kx, co] = w24q[8kx+ci, ky, co]
            with nc.allow_non_contiguous_dma(reason="one-time w rearrange"):
                for kx in range(3):
                    nc.scalar.dma_start(
                        out=qw_t[:, kx::3, :],
                        in_=w24q[8 * kx:8 * kx + 8, :, :],
                    )
                nc.vector.memset(qwbig[:, :, :], 0.0)
                for i in range(IMG):
                    nc.scalar.dma_start(
                        out=qwbig[8 * i:8 * i + 8, :, 8 * i:8 * i + 8],
                        in_=qw_t[:, :, :],
                    )

            # bias vector on evac partitions p = 8*img + co: log-doubling
            nc.scalar.dma_start(out=bias_e[0:CO, :], in_=b_ext[:])
            for m in (8, 16, 32, 64):
                nc.scalar.dma_start(out=bias_e[m:2 * m, :], in_=bias_e[0:m, :])

            # zero the pad columns of the qx stream (quantize never writes them)
            nc.vector.memset(qxh[:, :, 0:1], 0.0)
            nc.vector.memset(qxh[:, :, WP - 1:WP], 0.0)

            nc.vector.tensor_reduce(
                axg0[:, :], axg8[:, :], mybir.AxisListType.X,
                mybir.AluOpType.max,
            )
            nc.gpsimd.partition_broadcast(axg[:, :], axg0[:, :])

            # sx = 127/axg  (per-partition, all equal)
            nc.vector.reciprocal(sx[:, :], axg[:, :])
            nc.vector.tensor_scalar(
                out=sx[:, :], in0=sx[:, :], scalar1=MAX_Q, scalar2=None,
                op0=mybir.AluOpType.mult,
            )
            # inv = axg * aw / 127^2
            nc.gpsimd.partition_broadcast(aw128[:, :], aw_all[0:1, :])
            nc.vector.tensor_tensor(
                out=inv[:, :], in0=axg[:, :], in1=aw128[:, :],
                op=mybir.AluOpType.mult,
            )
            nc.vector.tensor_scalar(
                out=inv[:, :], in0=inv[:, :], scalar1=1.0 / (MAX_Q * MAX_Q),
                scalar2=None, op0=mybir.AluOpType.mult,
            )

            # ---------------- quantize ----------------
            # DVE: xh = xh*sx + MAGIC (in place); ACT: qxh = Copy(xh - MAGIC)
            def quant(r0, n, q0):
                nc.vector.tensor_scalar(
                    out=xh[:, r0:r0 + n, :], in0=xh[:, r0:r0 + n, :],
                    scalar1=sx[:, :], scalar2=MAGIC,
                    op0=mybir.AluOpType.mult, op1=mybir.AluOpType.add,
                )
                nc.scalar.activation(
                    qxh[:, q0 + r0:q0 + r0 + n, 1:W + 1], xh[:, r0:r0 + n, :],
                    mybir.ActivationFunctionType.Copy, bias=-MAGIC, scale=1.0,
                )

            for (r0, n) in ((0, 4), (4, 20), (24, 26)):   # h0 resident in xh
                quant(r0, n, 0)
            # reload h1 on two queues, quantize as pieces land
            load_piece(nc.sync, 0, 50, 24)
            quant(0, 24, 50)
            load_piece(nc.gpsimd, 24, 74, 24)
            quant(24, 24, 50)

            # ---------------- conv: 9 taps x 73 banks ----------------
            evac_cycle = (nc.scalar, nc.vector)
            out_q = (nc.sync, nc.gpsimd)
            bank = 0
            g_out = 0
            group_sizes = [GROUP] * 8 + [5, 3, 1]
            assert sum(group_sizes) == NBANK
            while bank < NBANK:
                nb = group_sizes[g_out]
                glen = sum(
                    min(512, S_END - (S_BEG + 512 * (bank + k))) for k in range(nb)
                )
                st = spool.tile([128, GROUP * 512], f32, tag="st")
                for k in range(nb):
                    b = bank + k
                    s_a = S_BEG + 512 * b
                    N = min(512, S_END - s_a)
                    ps = pspool.tile([128, 512], f32, tag="ps")
                    for g9 in range(9):
                        ky, kx = divmod(g9, 3)
                        off = s_a + ky * WP + kx - 1
                        nc.tensor.matmul(
                            ps[:, 0:N],
                            qwbig_f[:, 128 * g9:128 * g9 + 128],
                            qxh_f[:, off:off + N],
                            start=(g9 == 0),
                            stop=(g9 == 8),
                        )
                    eng = evac_cycle[b % 2]
                    if eng is nc.scalar:
                        nc.scalar.activation(
                            st[:, 512 * k:512 * k + N], ps[:, 0:N],
                            mybir.ActivationFunctionType.Identity,
                            bias=bias_e[:, :], scale=inv[:, :],
                        )
                    else:
                        eng.tensor_scalar(
                            out=st[:, 512 * k:512 * k + N], in0=ps[:, 0:N],
                            scalar1=inv[:, :], scalar2=bias_e[:, :],
                            op0=mybir.AluOpType.mult, op1=mybir.AluOpType.add,
                        )
                s_g = S_BEG + 512 * bank
                out_q[g_out % 2].dma_start(
                    out=out_ext[:, :, s_g:s_g + glen], in_=st[:, 0:glen]
                )
                bank += nb
                g_out += 1

    nc.compile()
    return nc


def _get_nc():
    if "nc" not in _cached:
        _cached["nc"] = _build()
    return _cached["nc"]


def make_core_inputs(x, weight, bias):
    """Shard full inputs into per-core input maps (host side)."""
    x = np.ascontiguousarray(x, dtype=np.float32)
    weight = np.ascontiguousarray(weight, dtype=np.float32)
    bias = np.ascontiguousarray(bias, dtype=np.float32)
    in_maps = []
    for core in range(N_CORES):
        b, h = divmod(core, 4)
        xc = np.zeros((IMG, CI, RH, W), dtype=np.float32)
        lo = 96 * h - 1
        src_lo, src_hi = max(lo, 0), min(lo + RH, H)
        xc[:, :, src_lo - lo:src_hi - lo, :] = (
            x[IMG * b:IMG * b + IMG, :, src_lo:src_hi, :]
        )
        in_maps.append({"x": xc, "weight": weight, "bias": bias})
    return in_maps


def assemble_output(results):
    """Gather per-core padded streams into the full output."""
    out = np.empty((2 * IMG, CO, H, W), dtype=np.float32)
    for core in range(N_CORES):
        b, h = divmod(core, 4)
        strip = results[core]["out"].reshape(IMG, CO, ROWS, WP)[:, :, :, 1:W + 1]
        out[IMG * b:IMG * b + IMG, :, 96 * h:96 * h + ROWS, :] = strip
    return out


def kernel(x, weight, bias):
    from concourse.bass_utils import run_bass_kernel_spmd

    nc = _get_nc()
    in_maps = make_core_inputs(x, weight, bias)
    res = run_bass_kernel_spmd(nc, in_maps, core_ids=list(range(N_CORES)))
    return assemble_output(res.results)
